# revision 23
# baseline (speedup 1.0000x reference)
"""Trainium2 Bass kernel for nn_BaselineAttention (B=2, N=2048, IN=512, D=1024, H=16, V=1).

Math folds (host-side, exact):
  A_h = Wq[h] @ Wk[h].T / sqrt(D)   ->  scores = h @ A_h @ h.T
  M   = Wo @ dec_w                  ->  out    = ctx @ M + dec_b
This removes the K projection, the Wo partial + decode GEMM, and ALL
collectives.

Sharding: core c -> batch c//4, query rows qs = 512*(c%4) .. +512, ALL 16
heads. Each core is fully independent (no cross-core communication):
  hT  = emb_w.T @ xT + cT            [D, N]    (f32r, full batch, K side)
  hQ  = emb_w.T @ xq + cq            [D, 512]  (bf16, q-slice)
  per head h:
    gT = A_h.T-contraction vs hQ     [D, 512]  (bf16 x bf16 -> f32r)
    scores tile = gT.T @ hT          [128q, 512k] f32r x f32r psum,
      eb-major over 4 open psum banks (stationary reused 4x),
      col 0 zeroed (multiplicative mask), p = exp(scores) (bf16),
      row-sums via ACT accum_out
    ctx = (p . V) / (p . 1)  via DVE mult + reduce (V broadcast via DRAM)
  out rows = ctxT.T @ M + dec_b      [512, 1024]  (bf16 x bf16)
Host reassembles the 8 [512, 1024] shards into [2, 2048, 1024].

Timing note: build(loop_k=K) wraps the body in a hardware For_i so one
dispatch runs the kernel K times (amortizes the ~70-120ms axon-tunnel
dispatch floor); test.py reports min_wall/K.
"""
import numpy as np

import concourse.bass as bass
import concourse.mybir as mybir
import concourse.tile as tile
from concourse import bacc
from concourse.bass_utils import run_bass_kernel_spmd
from concourse.masks import make_identity

F32 = mybir.dt.float32
F32R = mybir.dt.float32r
BF16 = mybir.dt.bfloat16
AX = mybir.AxisListType
OP = mybir.AluOpType
ACTF = mybir.ActivationFunctionType

N_CORES = 8
B, N, IN, D, H, NCLS = 2, 2048, 512, 1024, 16, 1024
P = 128
DC = D // P          # 8 d-chunks
IC = IN // P         # 4 in-chunks
KB = N // 512        # 4 k-blocks of 512
QS = 512             # q rows per core
QG = QS // P         # 4 q-tiles of 128
SCALE = 1.0 / np.sqrt(np.float32(D))


def build(loop_k: int = 1):
    nc = bacc.Bacc("TRN2", target_bir_lowering=False, debug=False, num_devices=N_CORES)

    xT = nc.dram_tensor("xT", [IN, N], F32R, kind="ExternalInput").ap()
    xq = nc.dram_tensor("xq", [IN, QS], F32R, kind="ExternalInput").ap()
    cT = nc.dram_tensor("cT", [D, N], F32, kind="ExternalInput").ap()
    cq = nc.dram_tensor("cq", [D, QS], F32, kind="ExternalInput").ap()
    emb_w = nc.dram_tensor("emb_w", [IN, D], F32R, kind="ExternalInput").ap()
    a_all = nc.dram_tensor("a_all", [H, D, D], BF16, kind="ExternalInput").ap()
    wv = nc.dram_tensor("wv", [D, H], F32R, kind="ExternalInput").ap()
    mo = nc.dram_tensor("mo", [P, NCLS], BF16, kind="ExternalInput").ap()
    dec_bb = nc.dram_tensor("dec_bb", [P, NCLS], F32, kind="ExternalInput").ap()
    out = nc.dram_tensor("out", [QS, NCLS], F32, kind="ExternalOutput").ap()

    from contextlib import ExitStack

    with tile.TileContext(nc) as tc:
        with ExitStack() as es:
            big = es.enter_context(tc.tile_pool(name="big", bufs=1))
            wpool = es.enter_context(tc.tile_pool(name="w", bufs=2))
            gtp = es.enter_context(tc.tile_pool(name="gt", bufs=1))
            ppool = es.enter_context(tc.tile_pool(name="pp", bufs=2))
            pscrp = es.enter_context(tc.tile_pool(name="pscr", bufs=1))
            vbp = es.enter_context(tc.tile_pool(name="vbp", bufs=1))
            xtp = es.enter_context(tc.tile_pool(name="xt", bufs=2))
            ctp = es.enter_context(tc.tile_pool(name="ct", bufs=2))
            cst = es.enter_context(tc.tile_pool(name="cst", bufs=1))
            stp = es.enter_context(tc.tile_pool(name="st", bufs=2))
            ctxTp = es.enter_context(tc.tile_pool(name="ctxT", bufs=2))
            finp = es.enter_context(tc.tile_pool(name="fin", bufs=2))
            scp = es.enter_context(tc.tile_pool(name="sc", bufs=5, space="PSUM"))
            accp = es.enter_context(tc.tile_pool(name="acc", bufs=2, space="PSUM"))
            dram = es.enter_context(tc.tile_pool(name="dram", bufs=1, space="DRAM"))
            vt_dram = dram.tile([H, N], BF16)

            ident = cst.tile([P, P], F32, tag="ident")
            make_identity(nc, ident)
            dbb = cst.tile([P, NCLS], F32, tag="dbb")
            nc.sync.dma_start(dbb[:], dec_bb[:])
            mo_sb = cst.tile([P, NCLS], BF16, tag="mo")
            nc.sync.dma_start(mo_sb[:], mo[:])
            wv_sb = cst.tile([P, DC, H], F32R, tag="wv")
            nc.sync.dma_start(wv_sb[:], wv.rearrange("(dc p) h -> p dc h", p=P))
            vT = cst.tile([H, N], BF16, tag="vT")
            ctxh = cst.tile([P, QG, H], F32, tag="ctxh")

            import contextlib

            loop_cm = (
                tc.For_i(0, loop_k, 1) if loop_k > 1 else contextlib.nullcontext()
            )
            with loop_cm:
              hT = big.tile([P, DC, N], F32R, tag="hT")
              hQ = big.tile([P, DC, QS], BF16, tag="hQ")

              # ---- embedding (full batch, K side): hT[dc, n]
              embw = wpool.tile([P, IC, D], F32R, tag="w")
              nc.sync.dma_start(embw[:], emb_w.rearrange("(ic p) d -> p ic d", p=P))
              for nch in range(4):
                  xt = xtp.tile([P, IC, 512], F32R)
                  nc.sync.dma_start(
                      xt[:], xT[:, nch * 512 : (nch + 1) * 512].rearrange(
                          "(ic p) n -> p ic n", p=P)
                  )
                  for dc in range(DC):
                      ps = accp.tile([P, 512], F32, tag="acc")
                      for ic in range(IC):
                          nc.tensor.matmul(
                              ps[:], embw[:, ic, dc * P : (dc + 1) * P],
                              xt[:, ic, :], start=(ic == 0), stop=(ic == IC - 1),
                          )
                      ct = ctp.tile([P, 512], F32)
                      nc.sync.dma_start(
                          ct[:], cT[dc * P : (dc + 1) * P, nch * 512 : (nch + 1) * 512]
                      )
                      nc.vector.tensor_tensor(
                          hT[:, dc, nch * 512 : (nch + 1) * 512], ps[:], ct[:], OP.add
                      )

              # ---- embedding (q-slice): hQ[dc, q]
              xqt = xtp.tile([P, IC, 512], F32R)
              nc.sync.dma_start(xqt[:], xq.rearrange("(ic p) n -> p ic n", p=P))
              for dc in range(DC):
                  ps = accp.tile([P, 512], F32, tag="acc")
                  for ic in range(IC):
                      nc.tensor.matmul(
                          ps[:], embw[:, ic, dc * P : (dc + 1) * P],
                          xqt[:, ic, :], start=(ic == 0), stop=(ic == IC - 1),
                      )
                  cqt = ctp.tile([P, 512], F32)
                  nc.sync.dma_start(cqt[:], cq[dc * P : (dc + 1) * P, :])
                  nc.vector.tensor_tensor(hQ[:, dc, :], ps[:], cqt[:], OP.add)

              # ---- V^T for all heads: vT[h, n] = sum_d wv[d, h] * hT[d, n]
              for nch in range(4):
                  pv = accp.tile([H, 512], F32, tag="acc")
                  for dc in range(DC):
                      nc.tensor.matmul(
                          pv[:], wv_sb[:, dc, :],
                          hT[:, dc, nch * 512 : (nch + 1) * 512],
                          start=(dc == 0), stop=(dc == DC - 1),
                      )
                  nc.scalar.copy(vT[:, nch * 512 : (nch + 1) * 512], pv[:])
              nc.sync.dma_start(vt_dram[:], vT[:])

              for hh in range(H):
                  # gT[e, q] = sum_d A[d, e] * hQ[d, q]   (A pre-scaled by 1/sqrt(D))
                  amat = wpool.tile([P, DC, D], BF16, tag="w")
                  nc.sync.dma_start(amat[:], a_all[hh].rearrange("(dc p) e -> p dc e", p=P))
                  gT = gtp.tile([P, DC, QS], F32R, tag="gT")
                  for eb in range(DC):
                      pg = accp.tile([P, 512], F32, tag="acc")
                      for dc in range(DC):
                          nc.tensor.matmul(
                              pg[:], amat[:, dc, eb * P : (eb + 1) * P],
                              hQ[:, dc, :], start=(dc == 0), stop=(dc == DC - 1),
                          )
                      nc.vector.tensor_scalar_add(gT[:, eb, :], pg[:], 0.0)

                  vb = vbp.tile([P, N], BF16, tag="vb")
                  nc.sync.dma_start(vb[:], vt_dram[hh].partition_broadcast(P))

                  for g in range(QG):
                      p_t = ppool.tile([P, N], BF16)
                      sts = stp.tile([P, 8], F32)
                      # eb-major: the stationary gT slice is reused across the
                      # 4 open k-block psum groups (4x fewer PE weight loads)
                      pss = [
                          scp.tile([P, 512], F32, tag="sc", name=f"ps{kb}")
                          for kb in range(KB)
                      ]
                      for eb in range(DC):
                          for kb in range(KB):
                              nc.tensor.matmul(
                                  pss[kb][:], gT[:, eb, g * P : (g + 1) * P],
                                  hT[:, eb, kb * 512 : (kb + 1) * 512],
                                  start=(eb == 0), stop=(eb == DC - 1),
                              )
                      for kb in range(KB):
                          ps = pss[kb]
                          if kb == 0:
                              nc.vector.memset(ps[:, 0:1], 0.0)
                          nc.scalar.activation(
                              p_t[:, kb * 512 : (kb + 1) * 512], ps[:], ACTF.Exp,
                              bias=0.0, scale=1.0,
                              accum_out=sts[:, kb : kb + 1],
                          )
                      scr = pscrp.tile([P, N], BF16, tag="scr")
                      nc.vector.tensor_tensor(scr[:], p_t[:], vb[:], OP.mult)
                      nc.vector.tensor_reduce(
                          sts[:, 4:5], scr[:], axis=AX.X, op=OP.add
                      )
                      nc.vector.tensor_reduce(
                          sts[:, 5:6], sts[:, 0:4], axis=AX.X, op=OP.add
                      )
                      nc.vector.reciprocal(sts[:, 6:7], sts[:, 5:6])
                      nc.vector.tensor_tensor(
                          ctxh[:, g, hh : hh + 1], sts[:, 4:5], sts[:, 6:7], OP.mult
                      )

              # ---- out rows = ctx @ M + dec_b
              for g in range(QG):
                  tp = accp.tile([H, P], F32, tag="acc")
                  nc.tensor.transpose(tp[:], ctxh[:, g, :], ident[:])
                  ctxT = ctxTp.tile([P, P], BF16)
                  nc.vector.memset(ctxT[:], 0.0)
                  nc.scalar.copy(ctxT[0:H, :], tp[:])
                  for j in range(2):
                      pd = accp.tile([P, 512], F32, tag="acc")
                      nc.tensor.matmul(
                          pd[:], ctxT[:], mo_sb[:, j * 512 : (j + 1) * 512],
                          start=True, stop=True,
                      )
                      fin = finp.tile([P, 512], F32)
                      nc.vector.tensor_tensor(
                          fin[:], pd[:], dbb[:, j * 512 : (j + 1) * 512], OP.add
                      )
                      nc.sync.dma_start(
                          out[g * P : (g + 1) * P, j * 512 : (j + 1) * 512], fin[:]
                      )
    nc.compile()
    return nc


_NC = {}


def _get_nc(loop_k: int = 1):
    if loop_k not in _NC:
        _NC[loop_k] = build(loop_k)
    return _NC[loop_k]


def _pos_encoding():
    pos = np.arange(N, dtype=np.float32)[:, None]
    div = np.exp(
        np.arange(0, D, 2, dtype=np.float32) * np.float32(-np.log(10000.0) / D)
    ).astype(np.float32)
    pe = np.zeros((N, D), dtype=np.float32)
    pe[:, 0::2] = np.sin(pos * div)
    pe[:, 1::2] = np.cos(pos * div)
    return pe


def make_in_maps(X, emb_w, emb_b, Wq, Wk, Wv, Wo, dec_w, dec_b):
    pe = _pos_encoding()
    emb_w = np.ascontiguousarray(emb_w, dtype=np.float32)
    cT = np.ascontiguousarray((pe + emb_b[None, :]).T.astype(np.float32))
    # A_h = Wq[h] @ Wk[h].T / sqrt(D)  (fold attention scale into A), bf16
    import ml_dtypes
    a_all = np.ascontiguousarray(
        (np.matmul(Wq.astype(np.float32), Wk.astype(np.float32).transpose(0, 2, 1))
         * np.float32(SCALE)).astype(ml_dtypes.bfloat16)
    )
    # M = Wo @ dec_w, zero-padded to 128 contraction rows, bf16
    import ml_dtypes
    mo = np.zeros((P, NCLS), dtype=ml_dtypes.bfloat16)
    mo[:H] = (Wo.astype(np.float32) @ dec_w.astype(np.float32)).astype(
        ml_dtypes.bfloat16
    )
    wv2 = np.ascontiguousarray(Wv[:, :, 0].T.astype(np.float32))  # [D, H]
    dec_bb = np.ascontiguousarray(
        np.broadcast_to(dec_b.astype(np.float32), (P, NCLS))
    )
    in_maps = []
    for c in range(N_CORES):
        b = c // 4
        qs = (c % 4) * QS
        xTb = np.ascontiguousarray(X[b].T.astype(np.float32))
        in_maps.append({
            "xT": xTb,
            "xq": np.ascontiguousarray(xTb[:, qs : qs + QS]),
            "cT": cT,
            "cq": np.ascontiguousarray(cT[:, qs : qs + QS]),
            "emb_w": emb_w,
            "a_all": a_all,
            "wv": wv2,
            "mo": mo,
            "dec_bb": dec_bb,
        })
    return in_maps


def run(trace=False, loop_k=1, **inputs):
    nc = _get_nc(loop_k)
    in_maps = make_in_maps(**inputs)
    res = run_bass_kernel_spmd(
        nc, in_maps, core_ids=list(range(N_CORES)), trace=trace
    )
    full = np.empty((B, N, NCLS), dtype=np.float32)
    for c in range(N_CORES):
        full[c // 4, (c % 4) * QS : (c % 4 + 1) * QS, :] = res.results[c]["out"]
    return full, res


def kernel(**inputs):
    full, _ = run(trace=False, **inputs)
    return full


def bench(iters=10, loop_k=1, nc=None, **inputs):
    """Time on-device NEFF execution (device-resident inputs, no donation)."""
    import time

    import jax
    import concourse.mybir as _mybir
    from concourse import bass2jax as b2j
    from jax.sharding import Mesh, PartitionSpec, NamedSharding
    from jax.experimental.shard_map import shard_map

    if nc is None:
        nc = _get_nc(loop_k)
    in_maps = make_in_maps(**inputs)
    b2j.install_neuronx_cc_hook()

    in_names, out_names, out_avals, zero_outs = [], [], [], []
    for alloc in nc.m.functions[0].allocations:
        if not isinstance(alloc, _mybir.MemoryLocationSet):
            continue
        name = alloc.memorylocations[0].name
        if alloc.kind == "ExternalInput":
            if not nc.partition_id_tensor or name != nc.partition_id_tensor.name:
                in_names.append(name)
        elif alloc.kind == "ExternalOutput":
            shape = tuple(alloc.tensor_shape)
            dtype = _mybir.dt.np(alloc.dtype)
            out_names.append(name)
            out_avals.append(jax.core.ShapedArray(shape, dtype))
            zero_outs.append(np.zeros(shape, dtype))
    n_params = len(in_names)
    all_in = list(in_names) + list(out_names)
    if nc.partition_id_tensor:
        all_in.append(nc.partition_id_tensor.name)

    def _body(*args):
        operands = list(args)
        if nc.partition_id_tensor:
            operands.append(b2j.partition_id_tensor())
        return tuple(
            b2j._bass_exec_p.bind(
                *operands,
                out_avals=tuple(out_avals),
                in_names=tuple(all_in),
                out_names=tuple(out_names),
                lowering_input_output_aliases=(),
                sim_require_finite=True,
                sim_require_nnan=True,
                nc=nc,
            )
        )

    devices = jax.devices()[:N_CORES]
    mesh = Mesh(np.asarray(devices), ("core",))
    nin = n_params + len(out_names)
    sharded = jax.jit(
        shard_map(
            _body, mesh=mesh, in_specs=(PartitionSpec("core"),) * nin,
            out_specs=(PartitionSpec("core"),) * len(out_names), check_rep=False,
        ),
        keep_unused=True,
    )
    sh = NamedSharding(mesh, PartitionSpec("core"))
    dev_in = [
        jax.device_put(
            np.concatenate([np.asarray(in_maps[c][k]) for c in range(N_CORES)], 0), sh
        )
        for k in in_names
    ] + [
        jax.device_put(np.zeros((N_CORES * z.shape[0], *z.shape[1:]), z.dtype), sh)
        for z in zero_outs
    ]
    outs = sharded(*dev_in)
    jax.block_until_ready(outs)  # warmup/compile
    times = []
    for _ in range(iters):
        t0 = time.perf_counter()
        outs = sharded(*dev_in)
        jax.block_until_ready(outs)
        times.append(time.perf_counter() - t0)
    full = np.empty((B, N, NCLS), dtype=np.float32)
    o = np.asarray(outs[out_names.index("out")]).reshape(N_CORES, QS, NCLS)
    for c in range(N_CORES):
        full[c // 4, (c % 4) * QS : (c % 4 + 1) * QS, :] = o[c]
    return full, times


# revision 30
# speedup vs baseline: 1.2379x; 1.2379x over previous
"""Trainium2 Bass kernel for nn_BaselineAttention (B=2, N=2048, IN=512, D=1024, H=16, V=1).

Math folds (host-side, exact):
  A_h = Wq[h] @ Wk[h].T / sqrt(D)   ->  scores = h @ A_h @ h.T
  M   = Wo @ dec_w                  ->  out    = ctx @ M + dec_b
This removes the K projection, the Wo partial + decode GEMM, and ALL
collectives.

Sharding: core c -> batch c//4, query rows qs = 512*(c%4) .. +512, ALL 16
heads. Each core is fully independent (no cross-core communication):
  hT  = emb_w.T @ xT + cT            [D, N]    (f32r, full batch, K side)
  hQ  = emb_w.T @ xq + cq            [D, 512]  (bf16, q-slice)
  per head h:
    gT = A_h.T-contraction vs hQ     [D, 512]  (bf16 x bf16 -> f32r)
    scores tile = gT.T @ hT          [128q, 512k] f32r x f32r psum,
      eb-major over 4 open psum banks (stationary reused 4x),
      col 0 zeroed (multiplicative mask), p = exp(scores) (bf16),
      row-sums via ACT accum_out
    ctx = (p . V) / (p . 1)  via DVE mult + reduce (V broadcast via DRAM)
  out rows = ctxT.T @ M + dec_b      [512, 1024]  (bf16 x bf16)
Host reassembles the 8 [512, 1024] shards into [2, 2048, 1024].

Timing note: build(loop_k=K) wraps the body in a hardware For_i so one
dispatch runs the kernel K times (amortizes the ~70-120ms axon-tunnel
dispatch floor); test.py reports min_wall/K.
"""
import numpy as np

import concourse.bass as bass
import concourse.mybir as mybir
import concourse.tile as tile
from concourse import bacc
from concourse.bass_utils import run_bass_kernel_spmd
from concourse.masks import make_identity

F32 = mybir.dt.float32
F32R = mybir.dt.float32r
BF16 = mybir.dt.bfloat16
AX = mybir.AxisListType
OP = mybir.AluOpType
ACTF = mybir.ActivationFunctionType

N_CORES = 8
B, N, IN, D, H, NCLS = 2, 2048, 512, 1024, 16, 1024
P = 128
DC = D // P          # 8 d-chunks
IC = IN // P         # 4 in-chunks
KB = N // 512        # 4 k-blocks of 512
QS = 512             # q rows per core
QG = QS // P         # 4 q-tiles of 128
SCALE = 1.0 / np.sqrt(np.float32(D))


def build(loop_k: int = 1):
    nc = bacc.Bacc("TRN2", target_bir_lowering=False, debug=False, num_devices=N_CORES)

    xT = nc.dram_tensor("xT", [IN, N], F32R, kind="ExternalInput").ap()
    xq = nc.dram_tensor("xq", [IN, QS], F32R, kind="ExternalInput").ap()
    cT = nc.dram_tensor("cT", [D, N], F32, kind="ExternalInput").ap()
    cq = nc.dram_tensor("cq", [D, QS], F32, kind="ExternalInput").ap()
    # weights host-pre-rearranged to SBUF tile layout: per-partition rows are
    # fully contiguous, so each load is few big DMA descriptors
    emb_w = nc.dram_tensor("emb_w", [P, IC, D], F32R, kind="ExternalInput").ap()
    a_all = nc.dram_tensor("a_all", [H, P, DC, D], BF16, kind="ExternalInput").ap()
    wv = nc.dram_tensor("wv", [P, DC, H], F32R, kind="ExternalInput").ap()
    mo = nc.dram_tensor("mo", [P, NCLS], BF16, kind="ExternalInput").ap()
    dec_bb = nc.dram_tensor("dec_bb", [P, NCLS], F32, kind="ExternalInput").ap()
    out = nc.dram_tensor("out", [QS, NCLS], F32, kind="ExternalOutput").ap()

    from contextlib import ExitStack

    with tile.TileContext(nc) as tc:
        with ExitStack() as es:
            big = es.enter_context(tc.tile_pool(name="big", bufs=1))
            wpool = es.enter_context(tc.tile_pool(name="w", bufs=2))
            gtp = es.enter_context(tc.tile_pool(name="gt", bufs=1))
            ppool = es.enter_context(tc.tile_pool(name="pp", bufs=2))
            pscrp = es.enter_context(tc.tile_pool(name="pscr", bufs=1))
            vbp = es.enter_context(tc.tile_pool(name="vbp", bufs=1))
            xtp = es.enter_context(tc.tile_pool(name="xt", bufs=2))
            ctp = es.enter_context(tc.tile_pool(name="ct", bufs=2))
            cst = es.enter_context(tc.tile_pool(name="cst", bufs=1))
            stp = es.enter_context(tc.tile_pool(name="st", bufs=2))
            ctxTp = es.enter_context(tc.tile_pool(name="ctxT", bufs=2))
            finp = es.enter_context(tc.tile_pool(name="fin", bufs=2))
            scp = es.enter_context(tc.tile_pool(name="sc", bufs=5, space="PSUM"))
            accp = es.enter_context(tc.tile_pool(name="acc", bufs=2, space="PSUM"))
            dram = es.enter_context(tc.tile_pool(name="dram", bufs=1, space="DRAM"))
            vt_dram = dram.tile([H, N], BF16)

            ident = cst.tile([P, P], F32, tag="ident")
            make_identity(nc, ident)
            dbb = cst.tile([P, NCLS], F32, tag="dbb")
            nc.sync.dma_start(dbb[:], dec_bb[:])
            mo_sb = cst.tile([P, NCLS], BF16, tag="mo")
            nc.sync.dma_start(mo_sb[:], mo[:])
            wv_sb = cst.tile([P, DC, H], F32R, tag="wv")
            nc.sync.dma_start(wv_sb[:], wv[:])
            vT = cst.tile([H, N], BF16, tag="vT")
            ctxh = cst.tile([P, QG, H], F32, tag="ctxh")

            import contextlib

            loop_cm = (
                tc.For_i(0, loop_k, 1) if loop_k > 1 else contextlib.nullcontext()
            )
            with loop_cm:
              hT = big.tile([P, DC, N], F32R, tag="hT")
              hQ = big.tile([P, DC, QS], BF16, tag="hQ")

              # ---- embedding (full batch, K side): hT[dc, n]
              embw = wpool.tile([P, IC, D], F32R, tag="w")
              nc.sync.dma_start(embw[:], emb_w[:])
              for nch in range(4):
                  xt = xtp.tile([P, IC, 512], F32R)
                  nc.sync.dma_start(
                      xt[:], xT[:, nch * 512 : (nch + 1) * 512].rearrange(
                          "(ic p) n -> p ic n", p=P)
                  )
                  for dc in range(DC):
                      ps = accp.tile([P, 512], F32, tag="acc")
                      for ic in range(IC):
                          nc.tensor.matmul(
                              ps[:], embw[:, ic, dc * P : (dc + 1) * P],
                              xt[:, ic, :], start=(ic == 0), stop=(ic == IC - 1),
                          )
                      ct = ctp.tile([P, 512], F32)
                      nc.sync.dma_start(
                          ct[:], cT[dc * P : (dc + 1) * P, nch * 512 : (nch + 1) * 512]
                      )
                      nc.vector.tensor_tensor(
                          hT[:, dc, nch * 512 : (nch + 1) * 512], ps[:], ct[:], OP.add
                      )

              # ---- embedding (q-slice): hQ[dc, q]
              xqt = xtp.tile([P, IC, 512], F32R)
              nc.sync.dma_start(xqt[:], xq.rearrange("(ic p) n -> p ic n", p=P))
              for dc in range(DC):
                  ps = accp.tile([P, 512], F32, tag="acc")
                  for ic in range(IC):
                      nc.tensor.matmul(
                          ps[:], embw[:, ic, dc * P : (dc + 1) * P],
                          xqt[:, ic, :], start=(ic == 0), stop=(ic == IC - 1),
                      )
                  cqt = ctp.tile([P, 512], F32)
                  nc.sync.dma_start(cqt[:], cq[dc * P : (dc + 1) * P, :])
                  nc.vector.tensor_tensor(hQ[:, dc, :], ps[:], cqt[:], OP.add)

              # ---- V^T for all heads: vT[h, n] = sum_d wv[d, h] * hT[d, n]
              for nch in range(4):
                  pv = accp.tile([H, 512], F32, tag="acc")
                  for dc in range(DC):
                      nc.tensor.matmul(
                          pv[:], wv_sb[:, dc, :],
                          hT[:, dc, nch * 512 : (nch + 1) * 512],
                          start=(dc == 0), stop=(dc == DC - 1),
                      )
                  nc.scalar.copy(vT[:, nch * 512 : (nch + 1) * 512], pv[:])
              nc.sync.dma_start(vt_dram[:], vT[:])

              for hh in range(H):
                  # gT[e, q] = sum_d A[d, e] * hQ[d, q]   (A pre-scaled by 1/sqrt(D))
                  amat = wpool.tile([P, DC, D], BF16, tag="w")
                  nc.sync.dma_start(amat[:], a_all[hh])
                  gT = gtp.tile([P, DC, QS], F32R, tag="gT")
                  for eb in range(DC):
                      pg = accp.tile([P, 512], F32, tag="acc")
                      for dc in range(DC):
                          nc.tensor.matmul(
                              pg[:], amat[:, dc, eb * P : (eb + 1) * P],
                              hQ[:, dc, :], start=(dc == 0), stop=(dc == DC - 1),
                          )
                      nc.vector.tensor_scalar_add(gT[:, eb, :], pg[:], 0.0)

                  vb = vbp.tile([P, N], BF16, tag="vb")
                  nc.sync.dma_start(vb[:], vt_dram[hh].partition_broadcast(P))

                  for g in range(QG):
                      p_t = ppool.tile([P, N], BF16)
                      sts = stp.tile([P, 8], F32)
                      # eb-major: the stationary gT slice is reused across the
                      # 4 open k-block psum groups (4x fewer PE weight loads)
                      pss = [
                          scp.tile([P, 512], F32, tag="sc", name=f"ps{kb}")
                          for kb in range(KB)
                      ]
                      for eb in range(DC):
                          for kb in range(KB):
                              nc.tensor.matmul(
                                  pss[kb][:], gT[:, eb, g * P : (g + 1) * P],
                                  hT[:, eb, kb * 512 : (kb + 1) * 512],
                                  start=(eb == 0), stop=(eb == DC - 1),
                              )
                      for kb in range(KB):
                          ps = pss[kb]
                          if kb == 0:
                              nc.vector.memset(ps[:, 0:1], 0.0)
                          nc.scalar.activation(
                              p_t[:, kb * 512 : (kb + 1) * 512], ps[:], ACTF.Exp,
                              bias=0.0, scale=1.0,
                              accum_out=sts[:, kb : kb + 1],
                          )
                      scr = pscrp.tile([P, N], BF16, tag="scr")
                      nc.vector.tensor_tensor(scr[:], p_t[:], vb[:], OP.mult)
                      nc.vector.tensor_reduce(
                          sts[:, 4:5], scr[:], axis=AX.X, op=OP.add
                      )
                      nc.vector.tensor_reduce(
                          sts[:, 5:6], sts[:, 0:4], axis=AX.X, op=OP.add
                      )
                      nc.vector.reciprocal(sts[:, 6:7], sts[:, 5:6])
                      nc.vector.tensor_tensor(
                          ctxh[:, g, hh : hh + 1], sts[:, 4:5], sts[:, 6:7], OP.mult
                      )

              # ---- out rows = ctx @ M + dec_b
              for g in range(QG):
                  tp = accp.tile([H, P], F32, tag="acc")
                  nc.tensor.transpose(tp[:], ctxh[:, g, :], ident[:])
                  ctxT = ctxTp.tile([P, P], BF16)
                  nc.vector.memset(ctxT[:], 0.0)
                  nc.scalar.copy(ctxT[0:H, :], tp[:])
                  for j in range(2):
                      pd = accp.tile([P, 512], F32, tag="acc")
                      nc.tensor.matmul(
                          pd[:], ctxT[:], mo_sb[:, j * 512 : (j + 1) * 512],
                          start=True, stop=True,
                      )
                      fin = finp.tile([P, 512], F32)
                      nc.vector.tensor_tensor(
                          fin[:], pd[:], dbb[:, j * 512 : (j + 1) * 512], OP.add
                      )
                      nc.sync.dma_start(
                          out[g * P : (g + 1) * P, j * 512 : (j + 1) * 512], fin[:]
                      )
    nc.compile()
    return nc


_NC = {}


def _get_nc(loop_k: int = 1):
    if loop_k not in _NC:
        _NC[loop_k] = build(loop_k)
    return _NC[loop_k]


def _pos_encoding():
    pos = np.arange(N, dtype=np.float32)[:, None]
    div = np.exp(
        np.arange(0, D, 2, dtype=np.float32) * np.float32(-np.log(10000.0) / D)
    ).astype(np.float32)
    pe = np.zeros((N, D), dtype=np.float32)
    pe[:, 0::2] = np.sin(pos * div)
    pe[:, 1::2] = np.cos(pos * div)
    return pe


def make_in_maps(X, emb_w, emb_b, Wq, Wk, Wv, Wo, dec_w, dec_b):
    pe = _pos_encoding()
    # emb_w [IN, D] -> [P, IC, D] (tile layout, contiguous per partition)
    emb_w = np.ascontiguousarray(
        np.asarray(emb_w, dtype=np.float32).reshape(IC, P, D).transpose(1, 0, 2)
    )
    cT = np.ascontiguousarray((pe + emb_b[None, :]).T.astype(np.float32))
    # A_h = Wq[h] @ Wk[h].T / sqrt(D)  (fold attention scale into A), bf16,
    # [H, D, D] -> [H, P, DC, D] tile layout
    import ml_dtypes
    a_all = (
        np.matmul(Wq.astype(np.float32), Wk.astype(np.float32).transpose(0, 2, 1))
        * np.float32(SCALE)
    ).astype(ml_dtypes.bfloat16)
    a_all = np.ascontiguousarray(
        a_all.reshape(H, DC, P, D).transpose(0, 2, 1, 3)
    )
    # M = Wo @ dec_w, zero-padded to 128 contraction rows, bf16
    import ml_dtypes
    mo = np.zeros((P, NCLS), dtype=ml_dtypes.bfloat16)
    mo[:H] = (Wo.astype(np.float32) @ dec_w.astype(np.float32)).astype(
        ml_dtypes.bfloat16
    )
    # Wv [H, D, 1] -> [D, H] -> [P, DC, H] tile layout
    wv2 = np.ascontiguousarray(
        Wv[:, :, 0].T.astype(np.float32).reshape(DC, P, H).transpose(1, 0, 2)
    )
    dec_bb = np.ascontiguousarray(
        np.broadcast_to(dec_b.astype(np.float32), (P, NCLS))
    )
    in_maps = []
    for c in range(N_CORES):
        b = c // 4
        qs = (c % 4) * QS
        xTb = np.ascontiguousarray(X[b].T.astype(np.float32))
        in_maps.append({
            "xT": xTb,
            "xq": np.ascontiguousarray(xTb[:, qs : qs + QS]),
            "cT": cT,
            "cq": np.ascontiguousarray(cT[:, qs : qs + QS]),
            "emb_w": emb_w,
            "a_all": a_all,
            "wv": wv2,
            "mo": mo,
            "dec_bb": dec_bb,
        })
    return in_maps


def run(trace=False, loop_k=1, **inputs):
    nc = _get_nc(loop_k)
    in_maps = make_in_maps(**inputs)
    res = run_bass_kernel_spmd(
        nc, in_maps, core_ids=list(range(N_CORES)), trace=trace
    )
    full = np.empty((B, N, NCLS), dtype=np.float32)
    for c in range(N_CORES):
        full[c // 4, (c % 4) * QS : (c % 4 + 1) * QS, :] = res.results[c]["out"]
    return full, res


def kernel(**inputs):
    full, _ = run(trace=False, **inputs)
    return full


def bench(iters=10, loop_k=1, nc=None, **inputs):
    """Time on-device NEFF execution (device-resident inputs, no donation)."""
    import time

    import jax
    import concourse.mybir as _mybir
    from concourse import bass2jax as b2j
    from jax.sharding import Mesh, PartitionSpec, NamedSharding
    from jax.experimental.shard_map import shard_map

    if nc is None:
        nc = _get_nc(loop_k)
    in_maps = make_in_maps(**inputs)
    b2j.install_neuronx_cc_hook()

    in_names, out_names, out_avals, zero_outs = [], [], [], []
    for alloc in nc.m.functions[0].allocations:
        if not isinstance(alloc, _mybir.MemoryLocationSet):
            continue
        name = alloc.memorylocations[0].name
        if alloc.kind == "ExternalInput":
            if not nc.partition_id_tensor or name != nc.partition_id_tensor.name:
                in_names.append(name)
        elif alloc.kind == "ExternalOutput":
            shape = tuple(alloc.tensor_shape)
            dtype = _mybir.dt.np(alloc.dtype)
            out_names.append(name)
            out_avals.append(jax.core.ShapedArray(shape, dtype))
            zero_outs.append(np.zeros(shape, dtype))
    n_params = len(in_names)
    all_in = list(in_names) + list(out_names)
    if nc.partition_id_tensor:
        all_in.append(nc.partition_id_tensor.name)

    def _body(*args):
        operands = list(args)
        if nc.partition_id_tensor:
            operands.append(b2j.partition_id_tensor())
        return tuple(
            b2j._bass_exec_p.bind(
                *operands,
                out_avals=tuple(out_avals),
                in_names=tuple(all_in),
                out_names=tuple(out_names),
                lowering_input_output_aliases=(),
                sim_require_finite=True,
                sim_require_nnan=True,
                nc=nc,
            )
        )

    devices = jax.devices()[:N_CORES]
    mesh = Mesh(np.asarray(devices), ("core",))
    nin = n_params + len(out_names)
    sharded = jax.jit(
        shard_map(
            _body, mesh=mesh, in_specs=(PartitionSpec("core"),) * nin,
            out_specs=(PartitionSpec("core"),) * len(out_names), check_rep=False,
        ),
        keep_unused=True,
    )
    sh = NamedSharding(mesh, PartitionSpec("core"))
    dev_in = [
        jax.device_put(
            np.concatenate([np.asarray(in_maps[c][k]) for c in range(N_CORES)], 0), sh
        )
        for k in in_names
    ] + [
        jax.device_put(np.zeros((N_CORES * z.shape[0], *z.shape[1:]), z.dtype), sh)
        for z in zero_outs
    ]
    outs = sharded(*dev_in)
    jax.block_until_ready(outs)  # warmup/compile
    times = []
    for _ in range(iters):
        t0 = time.perf_counter()
        outs = sharded(*dev_in)
        jax.block_until_ready(outs)
        times.append(time.perf_counter() - t0)
    full = np.empty((B, N, NCLS), dtype=np.float32)
    o = np.asarray(outs[out_names.index("out")]).reshape(N_CORES, QS, NCLS)
    for c in range(N_CORES):
        full[c // 4, (c % 4) * QS : (c % 4 + 1) * QS, :] = o[c]
    return full, times
